# revision 24
# baseline (speedup 1.0000x reference)
"""BitLinear (BitNet b1.58 ternary-weight linear) Trainium2 kernel, 8-core SPMD.

Reference computation:
    gamma = max(mean(|W|), 1e-8)
    QW    = clip(round(W / gamma), -1, 1)          # in {-1, 0, 1}
    out   = x @ QW.T + bias                        # x: [4, 2048, 4096] f32

Sharding (2 x 4 grid over 8 cores):
    - x   split in half along the (flattened) batch axis M=8192 -> M_loc=4096,
      transposed on host to xT [K, M_loc] so the contraction dim lands on
      SBUF partitions.
    - W   split in 4 along out_features N=4096 -> N_loc=1024, transposed on
      host to wT [K, N_loc].  Each W shard is held by 2 cores (the two m-halves).
    - gamma needs mean(|W|) over the FULL W: each core abs-sums a disjoint
      1/8 slice (wg), a 1-element AllReduce sums across cores.

Per-core dataflow (one iteration):
    1. gamma: stream wg [K, ng] f32, per-k-tile abs-reduce on VectorE,
       cross-partition sum via PE, 1-float AllReduce, th = gamma/2 broadcast
       to [128, 1] via PE.
    2. quantize: stream wT f32; produce q2 = 2*qw in bf16, split by columns
       across two engines:
         - ScalarE half: q2 = Sign(w - th) + Sign(w + th)   (two ACTs + DVE add)
         - VectorE half: q2 = 2*(w > th) - 2*(w < -th)      (two tensor_scalar
           with per-partition th + DVE subtract)
    3. matmul: x m-tiles are DMA-cast to exact bf16 by SWDGE (no engine work);
       out[m, n] = sum_k x[m,k] * q2[n,k] accumulated in f32 PSUM over 32
       k-tiles in two 512-wide halves (one PSUM bank each, 7 half-banks in
       flight).
    4. epilogue: ScalarE copies PSUM->SBUF with scale 0.5 (undoing the 2x in
       q2), VectorE adds bias, DMA out.

With rep > 1 the kernel body is software-pipelined at trace level: iteration
r+1's gamma / AllReduce / quantize are emitted in small quanta between
iteration r's m-tiles, positioned so each quantum's dependencies are already
met when its engine reaches it (the sequencers are in-order, so a premature
wait head-of-line-blocks everything behind it on that engine).  In steady
state the PE then runs m-tile matmuls back to back with no phase bubble.

kernel(**inputs) takes the full unsharded inputs and returns the full output.
Host work is layout only (transpose / slice / broadcast / concat); all
arithmetic runs on the NeuronCores.
"""

import numpy as np

N_CORES = 8
GRID_M, GRID_N = 2, 4          # core c -> (mi, ni) = (c // GRID_N, c % GRID_N)

B, S, K, N = 4, 2048, 4096, 4096
M = B * S                      # 8192
M_LOC = M // GRID_M            # 4096
N_LOC = N // GRID_N            # 1024
TJ = K // 128                  # 32 k-tiles
MM_N = 512                     # matmul moving free dim (one PSUM bank of f32)

TH_FLOOR = 0.5e-8              # = (1e-8)/2, the gamma floor folded into th


def split_multi_waits(nc, limit=1):
    """The walrus build in this container supports only `limit` sync-waits on
    CTRL-type (Drain/NoOp) instructions, but Tile's exit barrier attaches one
    wait per outstanding processor.  Split the extras onto preceding
    single-wait NOPs on the same engine (waits execute in issue order on the
    sequencer, so this is semantically identical)."""
    import concourse.mybir as mybir

    n_split = 0
    for f in nc.m.functions:
        for b in f.blocks:
            out_list = []
            changed = False
            for ins in b.instructions:
                si = getattr(ins, "sync_info", None)
                ow = list(si.on_wait) if (si is not None and si.on_wait) else []
                if len(ow) > limit:
                    for j, w in enumerate(ow[:-limit]):
                        nop = mybir.InstNoOp(name=f"{ins.name}-ws{j}")
                        nop.engine = ins.engine
                        nop.sync_info = mybir.SyncInfo(on_wait=[w], on_update=[])
                        out_list.append(nop)
                        n_split += 1
                    si.on_wait = ow[-limit:]
                    changed = True
                out_list.append(ins)
            if changed:
                b.instructions = out_list
    return n_split


def dedup_ldweights(nc):
    """Tile lowers every matmul into an explicit Ldweights + Matmult pair, so
    two consecutive matmuls sharing one stationary tile reload the PE array
    twice.  Drop an Ldweights when the instruction directly before it is a
    Matmult whose stationary operand is byte-identical and the Ldweights
    carries no semaphore waits/updates — the weights are already in the
    array."""
    n_drop = 0
    for f in nc.m.functions:
        for b in f.blocks:
            insts = list(b.instructions)
            out_list = []
            for ins in insts:
                if (type(ins).__name__ == "InstLdweights"
                        and out_list
                        and type(out_list[-1]).__name__ == "InstMatmult"
                        and len(out_list[-1].ins) >= 2
                        and str(out_list[-1].ins[1]) == str(ins.ins[0])
                        and not (ins.sync_info and ins.sync_info.on_wait)
                        and not (ins.sync_info and ins.sync_info.on_update)):
                    n_drop += 1
                    continue
                out_list.append(ins)
            if n_drop:
                b.instructions = out_list
    return n_drop


def build_nc(m_loc=M_LOC, k=K, n_loc=N_LOC, n_cores=N_CORES,
             grid_m=GRID_M, split_waits=True, rep=1, mm_n=MM_N,
             dedup_ldw=True, n_act=640, gch_tch=1, gch_bufs=8,
             qtmp_bufs=2,
             wch_tch=2, wch_bufs=2, xbf_bufs=3, osb_bufs=2, ps_bufs=8,
             x_cast="dma", gate_loads=True,
             # pipeline emission points (m-tile index in the PREVIOUS
             # iteration's phase B after which each quantum is emitted)
             pip_wg=2, pip_red=3, pip_fin1=11, pip_rd=14, pip_fin2=15,
             pip_wt=12, pip_qnt=16):
    """Build the per-core Bass graph (SPMD: identical on every core).

    rep > 1 unrolls + software-pipelines the kernel body `rep` times (same
    inputs / outputs each time) so steady-state per-iteration HW time can be
    measured without per-dispatch tunnel overhead.  rep=1 is the real kernel.
    """
    import concourse.bass as bass
    import concourse.mybir as mybir
    import concourse.tile as tile

    f32 = mybir.dt.float32
    bf16 = mybir.dt.bfloat16
    Alu = mybir.AluOpType
    Act = mybir.ActivationFunctionType

    tj = k // 128
    m_tiles = m_loc // 128
    ng = n_loc // grid_m            # gamma-slice width (disjoint across cores)
    # AR over the disjoint wg slices sums |W| exactly once; th = gamma/2
    th_scale = 1.0 / (2.0 * k * (n_loc * (n_cores // grid_m)))
    n_half = (n_loc + mm_n - 1) // mm_n
    assert n_loc % mm_n == 0
    n_gch = tj // gch_tch           # gamma chunks
    n_wch = tj // wch_tch           # quantize chunks

    nc = bass.Bass(num_devices=n_cores)
    # xt is host-pre-tiled: xt[mi, p, t*128+j] = x_loc[mi*128+j, t*128+p]
    # so each m-tile's load is one fully-contiguous [128, tj*128] block.
    xt = nc.dram_tensor("xt", [m_tiles, 128, tj * 128], f32,
                        kind="ExternalInput")
    wt = nc.dram_tensor("wt", [k, n_loc], f32, kind="ExternalInput")
    wg = nc.dram_tensor("wg", [k, ng], f32, kind="ExternalInput")
    biasb = nc.dram_tensor("biasb", [128, n_loc], f32, kind="ExternalInput")
    out = nc.dram_tensor("out", [m_loc, n_loc], f32, kind="ExternalOutput")

    cc_in = nc.dram_tensor("cc_in", [128], f32, kind="Internal")
    cc_out = nc.dram_tensor("cc_out", [128], f32, kind="Internal",
                            addr_space="Shared")

    wt_r = wt[:, :].rearrange("(t p) n -> p t n", p=128)
    wg_r = wg[:, :].rearrange("(t p) n -> p t n", p=128)

    with tile.TileContext(nc) as tc:
        with (
            tc.tile_pool(name="const", bufs=1) as constp,
            tc.tile_pool(name="gam", bufs=2) as gamp,
            tc.tile_pool(name="gch", bufs=gch_bufs) as gchp,
            tc.tile_pool(name="wch", bufs=wch_bufs) as wchp,
            tc.tile_pool(name="qtmp", bufs=qtmp_bufs) as qtmpp,
            tc.tile_pool(name="q2", bufs=2) as q2p,
            tc.tile_pool(name="xbf", bufs=xbf_bufs) as xbfp,
            tc.tile_pool(name="osb", bufs=osb_bufs) as osbp,
            tc.tile_pool(name="ps", bufs=ps_bufs, space="PSUM") as psp,
        ):
            # ---- constants ----
            biasb_sb = constp.tile([128, n_loc], f32, tag="biasb")
            nc.sync.dma_start(biasb_sb[:], biasb[:, :])

            # Per-iteration state carried between emit quanta.
            st = [dict() for _ in range(rep)]
            prev_cc = [None]          # last collective, for cc buffer reuse
            prev_rd = [None]          # last AR readback, for cc_out reuse

            def emit_gamma_wg(r, ci):
                """DMA one wg chunk (gch_tch k-tiles).  The SP ring carries
                only loads (out-stores ride the ACT ring), so these can never
                be head-of-line blocked by a store waiting on an epilogue."""
                s = st[r]
                gch = gchp.tile([128, gch_tch * ng], f32, tag="gch")
                gch3 = gch[:].rearrange("p (t n) -> p t n", n=ng)
                for t2 in range(gch_tch):
                    nc.sync.dma_start(gch3[:, t2, :],
                                      wg_r[:, ci * gch_tch + t2, :])
                s.setdefault("gch", []).append(gch3)

            def emit_gamma_red(r, ci):
                """Abs-reduce one wg chunk into acc[:, ...]."""
                s = st[r]
                if "acc" not in s:
                    acc = gamp.tile([128, tj], f32, tag="acc")
                    s["acc"] = acc
                gch3 = s["gch"][ci]
                for t2 in range(gch_tch):
                    t = ci * gch_tch + t2
                    red = nc.vector.tensor_reduce(
                        s["acc"][:, t:t + 1], gch3[:, t2, :],
                        axis=mybir.AxisListType.X, op=Alu.add,
                        apply_absolute_value=True)
                    if ci == 0 and t2 == 0 and r > 0 \
                            and "th_ins" in st[r - 1]:
                        # Keep this iteration's reduces behind the previous
                        # iteration's th on the in-order DVE stream — the
                        # scheduler otherwise hoists them in front, and a late
                        # AR then stalls th behind 30+ queued reduces.
                        tile.add_dep_helper(
                            red.ins, st[r - 1]["th_ins"].ins,
                            reason="gamma reds yield to previous th")

            def emit_gamma_fin1(r):
                """Final local reduce + ship the [128] per-partition partials
                to the AllReduce.  No PE involvement: a late collective can
                then never head-of-line block the matmul stream."""
                s = st[r]
                acc1 = gamp.tile([128, 1], f32, tag="acc1")
                nc.vector.tensor_reduce(acc1[:], s["acc"][:],
                                        axis=mybir.AxisListType.X, op=Alu.add)
                # cc DMAs ride the ACT HWDGE ring (not behind bulk SP loads)
                gate = nc.scalar.dma_start(cc_in[0:128], acc1[:, 0])
                if prev_cc[0] is not None:
                    tile.add_dep_helper(gate.ins, prev_cc[0].ins,
                                        reason="cc_in reuse after prior AR")
                cc = nc.gpsimd.collective_compute(
                    "AllReduce", Alu.add,
                    replica_groups=[list(range(n_cores))],
                    ins=[cc_in.ap().opt()], outs=[cc_out.ap().opt()])
                tile.add_dep_helper(cc.ins, gate.ins, reason="AR reads cc_in")
                if prev_rd[0] is not None:
                    tile.add_dep_helper(cc.ins, prev_rd[0].ins,
                                        reason="cc_out reuse after prior read")
                s["gate"], s["cc"] = gate, cc
                prev_cc[0] = cc

            def emit_gamma_rd(r):
                """Read the AR result back onto all 128 partitions via a
                0-stride broadcast DMA (emitted once the mesh is done so the
                ACT ring is never blocked waiting on it)."""
                s = st[r]
                s2b = gamp.tile([128, 128], f32, tag="s2b")
                rd = nc.scalar.dma_start(
                    s2b[:], cc_out[0:128].partition_broadcast(128))
                tile.add_dep_helper(rd.ins, s["cc"].ins,
                                    reason="read AR result")
                s["s2b"], s["rd"] = s2b, rd
                prev_rd[0] = rd

            def emit_gamma_fin2(r):
                """Reduce the broadcast AR vector, compute th/nth."""
                s = st[r]
                gsum = gamp.tile([128, 1], f32, tag="gsum")
                nc.vector.tensor_reduce(gsum[:], s["s2b"][:],
                                        axis=mybir.AxisListType.X, op=Alu.add)
                th = gamp.tile([128, 1], f32, tag="th")
                nth = gamp.tile([128, 1], f32, tag="nth")
                th_ins = nc.vector.tensor_scalar(
                    th[:], gsum[:], th_scale, TH_FLOOR,
                    op0=Alu.mult, op1=Alu.max)
                nc.vector.tensor_scalar(nth[:], gsum[:], -th_scale, -TH_FLOOR,
                                        op0=Alu.mult, op1=Alu.min)
                s["th"], s["nth"], s["th_ins"] = th, nth, th_ins

            def emit_wt(r, ci):
                """DMA one wt chunk (wch_tch k-tiles), gated so it can't
                starve the gamma stream of bandwidth."""
                s = st[r]
                wchs = s.setdefault("wch", {})
                wch = wchp.tile([128, wch_tch * n_loc], f32, tag="wg")
                wch3 = wch[:].rearrange("p (t n) -> p t n", n=n_loc)
                for t2 in range(wch_tch):
                    wd = nc.sync.dma_start(
                        wch3[:, t2, :], wt_r[:, ci * wch_tch + t2, :])
                    if gate_loads:
                        tile.add_dep_helper(
                            wd.ins, s["gate"].ins,
                            reason="wt loads yield to gamma stream")
                wchs[ci] = wch3

            def emit_quant(r, ci):
                """Quantize one wt chunk into q2 (split ScalarE / VectorE)."""
                s = st[r]
                if "q2" not in s:
                    q2 = q2p.tile([128, tj * n_loc], bf16, tag="q2")
                    s["q2"] = q2[:].rearrange("p (t n) -> p t n", n=n_loc)
                q2_3 = s["q2"]
                th, nth = s["th"], s["nth"]
                wch3 = s["wch"].pop(ci)
                n_dve = n_loc - n_act
                for t2 in range(wch_tch):
                    t = ci * wch_tch + t2
                    if n_act > 0:
                        a = qtmpp.tile([128, n_act], bf16, tag="qa")
                        b = qtmpp.tile([128, n_act], bf16, tag="qb")
                        nc.scalar.activation(
                            a[:], wch3[:, t2, 0:n_act], Act.Sign,
                            bias=nth[:], scale=1.0)
                        nc.scalar.activation(
                            b[:], wch3[:, t2, 0:n_act], Act.Sign,
                            bias=th[:], scale=1.0)
                        nc.vector.tensor_tensor(
                            q2_3[:, t, 0:n_act], a[:], b[:], op=Alu.add)
                    if n_dve > 0:
                        p2 = qtmpp.tile([128, n_dve], bf16, tag="qp")
                        m2 = qtmpp.tile([128, n_dve], bf16, tag="qm")
                        nc.vector.tensor_scalar(
                            p2[:], wch3[:, t2, n_act:], th[:], 2.0,
                            op0=Alu.is_gt, op1=Alu.mult)
                        nc.vector.tensor_scalar(
                            m2[:], wch3[:, t2, n_act:], nth[:], 2.0,
                            op0=Alu.is_lt, op1=Alu.mult)
                        nc.vector.tensor_tensor(
                            q2_3[:, t, n_act:], p2[:], m2[:], op=Alu.subtract)

            def emit_mtile(r, mi):
                """One m-tile of phase B: x load, 64 matmuls, epilogue."""
                s = st[r]
                xbf = xbfp.tile([128, tj * 128], bf16, tag="xbf")
                xbf3 = xbf[:].rearrange("p (t j) -> p t j", j=128)
                if x_cast == "dma":
                    xd = nc.gpsimd.dma_start(xbf[:], xt[mi, :, :])
                else:
                    xraw = xbfp.tile([128, tj * 128], f32, tag="xraw")
                    xd = nc.sync.dma_start(xraw[:], xt[mi, :, :])
                    nc.scalar.activation(xbf[:], xraw[:], Act.Copy, scale=1.0)
                if gate_loads:
                    tile.add_dep_helper(xd.ins, s["gate"].ins,
                                        reason="x loads yield to gamma stream")
                q2_3 = s["q2"]
                pss = []
                for h in range(n_half):
                    psh = psp.tile([128, mm_n], f32, tag="ps")
                    pss.append(psh)
                for t in range(tj):
                    for h in range(n_half):
                        n0 = h * mm_n
                        nc.tensor.matmul(pss[h][:],
                                         lhsT=xbf3[:, t, :],
                                         rhs=q2_3[:, t, n0:n0 + mm_n],
                                         start=(t == 0),
                                         stop=(t == tj - 1))
                osb = osbp.tile([128, n_loc], f32, tag="osb")
                for h in range(n_half):
                    n0 = h * mm_n
                    nc.scalar.activation(osb[:, n0:n0 + mm_n], pss[h][:],
                                         Act.Copy, scale=0.5)
                nc.vector.tensor_tensor(osb[:], osb[:], biasb_sb[:],
                                        op=Alu.add)
                # out-stores ride the ACT ring: on the SP ring their wait on
                # the epilogue would head-of-line block the next iteration's
                # gamma/wt loads.
                nc.scalar.dma_start(out[mi * 128:(mi + 1) * 128, :], osb[:])

            def emit_prologue(r):
                """Serial prologue (used for iteration 0 only)."""
                for ci in range(n_gch):
                    emit_gamma_wg(r, ci)
                    emit_gamma_red(r, ci)
                emit_gamma_fin1(r)
                emit_gamma_rd(r)
                emit_gamma_fin2(r)
                for ci in range(n_wch):
                    emit_wt(r, ci)
                for ci in range(n_wch):
                    emit_quant(r, ci)

            # Pipeline quanta for iteration r+1, keyed by the m-tile index of
            # iteration r's phase B after which they are emitted.  Dependencies
            # must already be met at that point in each engine's stream (the
            # sequencers are in-order, so a premature wait blocks everything
            # behind it on that engine).
            sched = {}
            for ci in range(n_gch):          # 4 wg chunks / m-tile
                sched.setdefault(pip_wg + ci // 4, []).append(("wg", ci))
            for ci in range(n_gch):
                sched.setdefault(pip_red + ci // 4, []).append(("red", ci))
            span_wt = m_tiles - 6 - pip_wt
            for ci in range(n_wch):          # wt chunks spread to ~m-tile 26
                sched.setdefault(
                    pip_wt + ci * span_wt // n_wch, []).append(("wt", ci))
            span_q = m_tiles - 2 - pip_qnt
            for ci in range(n_wch):          # quantize spread to ~m-tile 30
                sched.setdefault(
                    pip_qnt + ci * span_q // n_wch, []).append(("qnt", ci))
            sched.setdefault(pip_fin1, []).append(("fin1", None))
            sched.setdefault(pip_rd, []).append(("rd", None))
            sched.setdefault(pip_fin2, []).append(("fin2", None))

            emitters = {"wg": emit_gamma_wg, "red": emit_gamma_red,
                        "wt": emit_wt, "qnt": emit_quant,
                        "fin1": lambda r, ci: emit_gamma_fin1(r),
                        "rd": lambda r, ci: emit_gamma_rd(r),
                        "fin2": lambda r, ci: emit_gamma_fin2(r)}

            emit_prologue(0)
            for r in range(rep):
                pipelined = r + 1 < rep
                for mi in range(m_tiles):
                    emit_mtile(r, mi)
                    if pipelined:
                        for what, ci in sched.get(mi, []):
                            emitters[what](r + 1, ci)

    if dedup_ldw:
        dedup_ldweights(nc)
    if split_waits:
        split_multi_waits(nc)
    return nc


def shard_inputs(x, weight, bias, m_loc=M_LOC, n_loc=N_LOC, n_cores=N_CORES,
                 grid_n=GRID_N):
    """Host-side layout prep (transpose/slice/broadcast only)."""
    x2 = np.ascontiguousarray(x.reshape(-1, x.shape[-1]))     # [M, K]
    k = x2.shape[1]
    m_tiles, tj = m_loc // 128, k // 128
    grid_m = n_cores // grid_n
    ng = n_loc // grid_m
    in_maps = []
    xts = {}
    for c in range(n_cores):
        mi, ni = c // grid_n, c % grid_n
        if mi not in xts:
            # xt[mi, p, t*128+j] = x_loc[mi*128+j, t*128+p]
            xl = x2[mi * m_loc:(mi + 1) * m_loc, :]
            xts[mi] = np.ascontiguousarray(
                xl.reshape(m_tiles, 128, tj, 128)
                .transpose(0, 3, 2, 1)
                .reshape(m_tiles, 128, tj * 128))
        wt = np.ascontiguousarray(weight[ni * n_loc:(ni + 1) * n_loc, :].T)
        g0 = ni * n_loc + mi * ng
        wgt = np.ascontiguousarray(weight[g0:g0 + ng, :].T)
        bb = np.ascontiguousarray(
            np.broadcast_to(bias[ni * n_loc:(ni + 1) * n_loc], (128, n_loc)))
        in_maps.append({"xt": xts[mi], "wt": wt, "wg": wgt, "biasb": bb})
    return in_maps


def unshard_output(outs, x_shape, m_loc=M_LOC, n_loc=N_LOC, n_cores=N_CORES,
                   grid_m=GRID_M, grid_n=GRID_N):
    n = grid_n * n_loc
    full = np.empty((grid_m * m_loc, n), dtype=outs[0].dtype)
    for c in range(n_cores):
        mi, ni = c // grid_n, c % grid_n
        full[mi * m_loc:(mi + 1) * m_loc, ni * n_loc:(ni + 1) * n_loc] = outs[c]
    return full.reshape(*x_shape[:-1], n)


def kernel(x, weight, bias):
    from concourse.bass_utils import run_bass_kernel_spmd

    nc = build_nc()
    in_maps = shard_inputs(x, weight, bias)
    res = run_bass_kernel_spmd(nc, in_maps, core_ids=list(range(N_CORES)))
    outs = [res.results[c]["out"] for c in range(N_CORES)]
    return unshard_output(outs, x.shape)


# revision 27
# speedup vs baseline: 1.0106x; 1.0106x over previous
"""BitLinear (BitNet b1.58 ternary-weight linear) Trainium2 kernel, 8-core SPMD.

Reference computation:
    gamma = max(mean(|W|), 1e-8)
    QW    = clip(round(W / gamma), -1, 1)          # in {-1, 0, 1}
    out   = x @ QW.T + bias                        # x: [4, 2048, 4096] f32

Sharding (2 x 4 grid over 8 cores):
    - x   split in half along the (flattened) batch axis M=8192 -> M_loc=4096,
      transposed on host to xT [K, M_loc] so the contraction dim lands on
      SBUF partitions.
    - W   split in 4 along out_features N=4096 -> N_loc=1024, transposed on
      host to wT [K, N_loc].  Each W shard is held by 2 cores (the two m-halves).
    - gamma needs mean(|W|) over the FULL W: each core abs-sums a disjoint
      1/8 slice (wg), a 1-element AllReduce sums across cores.

Per-core dataflow (one iteration):
    1. gamma: stream wg [K, ng] f32, per-k-tile abs-reduce on VectorE,
       cross-partition sum via PE, 1-float AllReduce, th = gamma/2 broadcast
       to [128, 1] via PE.
    2. quantize: stream wT f32; produce q2 = 2*qw in bf16, split by columns
       across two engines:
         - ScalarE half: q2 = Sign(w - th) + Sign(w + th)   (two ACTs + DVE add)
         - VectorE half: q2 = 2*(w > th) - 2*(w < -th)      (two tensor_scalar
           with per-partition th + DVE subtract)
    3. matmul: x m-tiles are DMA-cast to exact bf16 by SWDGE (no engine work);
       out[m, n] = sum_k x[m,k] * q2[n,k] accumulated in f32 PSUM over 32
       k-tiles in two 512-wide halves (one PSUM bank each, 7 half-banks in
       flight).
    4. epilogue: ScalarE copies PSUM->SBUF with scale 0.5 (undoing the 2x in
       q2), VectorE adds bias, DMA out.

With rep > 1 the kernel body is software-pipelined at trace level: iteration
r+1's gamma / AllReduce / quantize are emitted in small quanta between
iteration r's m-tiles, positioned so each quantum's dependencies are already
met when its engine reaches it (the sequencers are in-order, so a premature
wait head-of-line-blocks everything behind it on that engine).  In steady
state the PE then runs m-tile matmuls back to back with no phase bubble.

kernel(**inputs) takes the full unsharded inputs and returns the full output.
Host work is layout only (transpose / slice / broadcast / concat); all
arithmetic runs on the NeuronCores.
"""

import numpy as np

N_CORES = 8
GRID_M, GRID_N = 2, 4          # core c -> (mi, ni) = (c // GRID_N, c % GRID_N)

B, S, K, N = 4, 2048, 4096, 4096
M = B * S                      # 8192
M_LOC = M // GRID_M            # 4096
N_LOC = N // GRID_N            # 1024
TJ = K // 128                  # 32 k-tiles
MM_N = 512                     # matmul moving free dim (one PSUM bank of f32)

TH_FLOOR = 0.5e-8              # = (1e-8)/2, the gamma floor folded into th


def split_multi_waits(nc, limit=1):
    """The walrus build in this container supports only `limit` sync-waits on
    CTRL-type (Drain/NoOp) instructions, but Tile's exit barrier attaches one
    wait per outstanding processor.  Split the extras onto preceding
    single-wait NOPs on the same engine (waits execute in issue order on the
    sequencer, so this is semantically identical)."""
    import concourse.mybir as mybir

    n_split = 0
    for f in nc.m.functions:
        for b in f.blocks:
            out_list = []
            changed = False
            for ins in b.instructions:
                si = getattr(ins, "sync_info", None)
                ow = list(si.on_wait) if (si is not None and si.on_wait) else []
                if len(ow) > limit:
                    for j, w in enumerate(ow[:-limit]):
                        nop = mybir.InstNoOp(name=f"{ins.name}-ws{j}")
                        nop.engine = ins.engine
                        nop.sync_info = mybir.SyncInfo(on_wait=[w], on_update=[])
                        out_list.append(nop)
                        n_split += 1
                    si.on_wait = ow[-limit:]
                    changed = True
                out_list.append(ins)
            if changed:
                b.instructions = out_list
    return n_split


def dedup_ldweights(nc):
    """Tile lowers every matmul into an explicit Ldweights + Matmult pair, so
    two consecutive matmuls sharing one stationary tile reload the PE array
    twice.  Drop an Ldweights when the instruction directly before it is a
    Matmult whose stationary operand is byte-identical and the Ldweights
    carries no semaphore waits/updates — the weights are already in the
    array."""
    n_drop = 0
    for f in nc.m.functions:
        for b in f.blocks:
            insts = list(b.instructions)
            out_list = []
            for ins in insts:
                if (type(ins).__name__ == "InstLdweights"
                        and out_list
                        and type(out_list[-1]).__name__ == "InstMatmult"
                        and len(out_list[-1].ins) >= 2
                        and str(out_list[-1].ins[1]) == str(ins.ins[0])
                        and not (ins.sync_info and ins.sync_info.on_wait)
                        and not (ins.sync_info and ins.sync_info.on_update)):
                    n_drop += 1
                    continue
                out_list.append(ins)
            if n_drop:
                b.instructions = out_list
    return n_drop


def build_nc(m_loc=M_LOC, k=K, n_loc=N_LOC, n_cores=N_CORES,
             grid_m=GRID_M, split_waits=True, rep=1, mm_n=MM_N,
             dedup_ldw=True, n_act=640, gch_tch=1, gch_bufs=6,
             qtmp_bufs=2,
             wch_tch=1, wch_bufs=5, xbf_bufs=3, osb_bufs=2, ps_bufs=8,
             x_cast="dma", gate_loads=True,
             # pipeline emission points (m-tile index in the PREVIOUS
             # iteration's phase B after which each quantum is emitted)
             pip_wg=2, pip_red=3, pip_fin1=11, pip_rd=14, pip_fin2=15,
             pip_wt=12, pip_qnt=16):
    """Build the per-core Bass graph (SPMD: identical on every core).

    rep > 1 unrolls + software-pipelines the kernel body `rep` times (same
    inputs / outputs each time) so steady-state per-iteration HW time can be
    measured without per-dispatch tunnel overhead.  rep=1 is the real kernel.
    """
    import concourse.bass as bass
    import concourse.mybir as mybir
    import concourse.tile as tile

    f32 = mybir.dt.float32
    bf16 = mybir.dt.bfloat16
    Alu = mybir.AluOpType
    Act = mybir.ActivationFunctionType

    tj = k // 128
    m_tiles = m_loc // 128
    ng = n_loc // grid_m            # gamma-slice width (disjoint across cores)
    # AR over the disjoint wg slices sums |W| exactly once; th = gamma/2
    th_scale = 1.0 / (2.0 * k * (n_loc * (n_cores // grid_m)))
    n_half = (n_loc + mm_n - 1) // mm_n
    assert n_loc % mm_n == 0
    n_gch = tj // gch_tch           # gamma chunks
    n_wch = tj // wch_tch           # quantize chunks

    nc = bass.Bass(num_devices=n_cores)
    # xt is host-pre-tiled: xt[mi, p, t*128+j] = x_loc[mi*128+j, t*128+p]
    # so each m-tile's load is one fully-contiguous [128, tj*128] block.
    xt = nc.dram_tensor("xt", [m_tiles, 128, tj * 128], f32,
                        kind="ExternalInput")
    wt = nc.dram_tensor("wt", [k, n_loc], f32, kind="ExternalInput")
    wg = nc.dram_tensor("wg", [k, ng], f32, kind="ExternalInput")
    biasb = nc.dram_tensor("biasb", [128, n_loc], f32, kind="ExternalInput")
    out = nc.dram_tensor("out", [m_loc, n_loc], f32, kind="ExternalOutput")

    cc_in = nc.dram_tensor("cc_in", [128], f32, kind="Internal")
    cc_out = nc.dram_tensor("cc_out", [128], f32, kind="Internal",
                            addr_space="Shared")

    wt_r = wt[:, :].rearrange("(t p) n -> p t n", p=128)
    wg_r = wg[:, :].rearrange("(t p) n -> p t n", p=128)

    with tile.TileContext(nc) as tc:
        with (
            tc.tile_pool(name="const", bufs=1) as constp,
            tc.tile_pool(name="gam", bufs=2) as gamp,
            tc.tile_pool(name="gch", bufs=gch_bufs) as gchp,
            tc.tile_pool(name="wch", bufs=wch_bufs) as wchp,
            tc.tile_pool(name="qtmp", bufs=qtmp_bufs) as qtmpp,
            tc.tile_pool(name="q2", bufs=2) as q2p,
            tc.tile_pool(name="xbf", bufs=xbf_bufs) as xbfp,
            tc.tile_pool(name="osb", bufs=osb_bufs) as osbp,
            tc.tile_pool(name="ps", bufs=ps_bufs, space="PSUM") as psp,
        ):
            # ---- constants ----
            biasb_sb = constp.tile([128, n_loc], f32, tag="biasb")
            nc.sync.dma_start(biasb_sb[:], biasb[:, :])

            # Per-iteration state carried between emit quanta.
            st = [dict() for _ in range(rep)]
            prev_cc = [None]          # last collective, for cc buffer reuse
            prev_rd = [None]          # last AR readback, for cc_out reuse

            def emit_gamma_wg(r, ci):
                """DMA one wg chunk (gch_tch k-tiles).  The SP ring carries
                only loads (out-stores ride the ACT ring), so these can never
                be head-of-line blocked by a store waiting on an epilogue."""
                s = st[r]
                gch = gchp.tile([128, gch_tch * ng], f32, tag="gch")
                gch3 = gch[:].rearrange("p (t n) -> p t n", n=ng)
                for t2 in range(gch_tch):
                    nc.sync.dma_start(gch3[:, t2, :],
                                      wg_r[:, ci * gch_tch + t2, :])
                s.setdefault("gch", []).append(gch3)

            def emit_gamma_red(r, ci):
                """Abs-reduce one wg chunk into acc[:, ...]."""
                s = st[r]
                if "acc" not in s:
                    acc = gamp.tile([128, tj], f32, tag="acc")
                    s["acc"] = acc
                gch3 = s["gch"][ci]
                for t2 in range(gch_tch):
                    t = ci * gch_tch + t2
                    red = nc.vector.tensor_reduce(
                        s["acc"][:, t:t + 1], gch3[:, t2, :],
                        axis=mybir.AxisListType.X, op=Alu.add,
                        apply_absolute_value=True)
                    if ci == 0 and t2 == 0 and r > 0 \
                            and "th_ins" in st[r - 1]:
                        # Keep this iteration's reduces behind the previous
                        # iteration's th on the in-order DVE stream — the
                        # scheduler otherwise hoists them in front, and a late
                        # AR then stalls th behind 30+ queued reduces.
                        tile.add_dep_helper(
                            red.ins, st[r - 1]["th_ins"].ins,
                            reason="gamma reds yield to previous th")

            def emit_gamma_fin1(r):
                """Final local reduce + ship the [128] per-partition partials
                to the AllReduce.  No PE involvement: a late collective can
                then never head-of-line block the matmul stream."""
                s = st[r]
                acc1 = gamp.tile([128, 1], f32, tag="acc1")
                nc.vector.tensor_reduce(acc1[:], s["acc"][:],
                                        axis=mybir.AxisListType.X, op=Alu.add)
                # cc DMAs ride the ACT HWDGE ring (not behind bulk SP loads)
                gate = nc.scalar.dma_start(cc_in[0:128], acc1[:, 0])
                if prev_cc[0] is not None:
                    tile.add_dep_helper(gate.ins, prev_cc[0].ins,
                                        reason="cc_in reuse after prior AR")
                cc = nc.gpsimd.collective_compute(
                    "AllReduce", Alu.add,
                    replica_groups=[list(range(n_cores))],
                    ins=[cc_in.ap().opt()], outs=[cc_out.ap().opt()])
                tile.add_dep_helper(cc.ins, gate.ins, reason="AR reads cc_in")
                if prev_rd[0] is not None:
                    tile.add_dep_helper(cc.ins, prev_rd[0].ins,
                                        reason="cc_out reuse after prior read")
                s["gate"], s["cc"] = gate, cc
                prev_cc[0] = cc

            def emit_gamma_rd(r):
                """Read the AR result back onto all 128 partitions via a
                0-stride broadcast DMA (emitted once the mesh is done so the
                ACT ring is never blocked waiting on it)."""
                s = st[r]
                s2b = gamp.tile([128, 128], f32, tag="s2b")
                rd = nc.scalar.dma_start(
                    s2b[:], cc_out[0:128].partition_broadcast(128))
                tile.add_dep_helper(rd.ins, s["cc"].ins,
                                    reason="read AR result")
                s["s2b"], s["rd"] = s2b, rd
                prev_rd[0] = rd

            def emit_gamma_fin2(r):
                """Reduce the broadcast AR vector, compute th/nth."""
                s = st[r]
                gsum = gamp.tile([128, 1], f32, tag="gsum")
                nc.vector.tensor_reduce(gsum[:], s["s2b"][:],
                                        axis=mybir.AxisListType.X, op=Alu.add)
                th = gamp.tile([128, 1], f32, tag="th")
                nth = gamp.tile([128, 1], f32, tag="nth")
                th_ins = nc.vector.tensor_scalar(
                    th[:], gsum[:], th_scale, TH_FLOOR,
                    op0=Alu.mult, op1=Alu.max)
                nc.vector.tensor_scalar(nth[:], gsum[:], -th_scale, -TH_FLOOR,
                                        op0=Alu.mult, op1=Alu.min)
                s["th"], s["nth"], s["th_ins"] = th, nth, th_ins

            def emit_wt(r, ci):
                """DMA one wt chunk (wch_tch k-tiles), gated so it can't
                starve the gamma stream of bandwidth."""
                s = st[r]
                wchs = s.setdefault("wch", {})
                wch = wchp.tile([128, wch_tch * n_loc], f32, tag="wg")
                wch3 = wch[:].rearrange("p (t n) -> p t n", n=n_loc)
                for t2 in range(wch_tch):
                    wd = nc.sync.dma_start(
                        wch3[:, t2, :], wt_r[:, ci * wch_tch + t2, :])
                    if gate_loads:
                        tile.add_dep_helper(
                            wd.ins, s["gate"].ins,
                            reason="wt loads yield to gamma stream")
                wchs[ci] = wch3

            def emit_quant(r, ci):
                """Quantize one wt chunk into q2 (split ScalarE / VectorE)."""
                s = st[r]
                if "q2" not in s:
                    q2 = q2p.tile([128, tj * n_loc], bf16, tag="q2")
                    s["q2"] = q2[:].rearrange("p (t n) -> p t n", n=n_loc)
                q2_3 = s["q2"]
                th, nth = s["th"], s["nth"]
                wch3 = s["wch"].pop(ci)
                n_dve = n_loc - n_act
                for t2 in range(wch_tch):
                    t = ci * wch_tch + t2
                    if n_act > 0:
                        a = qtmpp.tile([128, n_act], bf16, tag="qa")
                        b = qtmpp.tile([128, n_act], bf16, tag="qb")
                        nc.scalar.activation(
                            a[:], wch3[:, t2, 0:n_act], Act.Sign,
                            bias=nth[:], scale=1.0)
                        nc.scalar.activation(
                            b[:], wch3[:, t2, 0:n_act], Act.Sign,
                            bias=th[:], scale=1.0)
                        nc.vector.tensor_tensor(
                            q2_3[:, t, 0:n_act], a[:], b[:], op=Alu.add)
                    if n_dve > 0:
                        p2 = qtmpp.tile([128, n_dve], bf16, tag="qp")
                        m2 = qtmpp.tile([128, n_dve], bf16, tag="qm")
                        nc.vector.tensor_scalar(
                            p2[:], wch3[:, t2, n_act:], th[:], 2.0,
                            op0=Alu.is_gt, op1=Alu.mult)
                        nc.vector.tensor_scalar(
                            m2[:], wch3[:, t2, n_act:], nth[:], 2.0,
                            op0=Alu.is_lt, op1=Alu.mult)
                        nc.vector.tensor_tensor(
                            q2_3[:, t, n_act:], p2[:], m2[:], op=Alu.subtract)

            def emit_mtile(r, mi):
                """One m-tile of phase B: x load, 64 matmuls, epilogue."""
                s = st[r]
                xbf = xbfp.tile([128, tj * 128], bf16, tag="xbf")
                xbf3 = xbf[:].rearrange("p (t j) -> p t j", j=128)
                if x_cast == "dma":
                    xd = nc.gpsimd.dma_start(xbf[:], xt[mi, :, :])
                else:
                    xraw = xbfp.tile([128, tj * 128], f32, tag="xraw")
                    xd = nc.sync.dma_start(xraw[:], xt[mi, :, :])
                    nc.scalar.activation(xbf[:], xraw[:], Act.Copy, scale=1.0)
                if gate_loads:
                    tile.add_dep_helper(xd.ins, s["gate"].ins,
                                        reason="x loads yield to gamma stream")
                q2_3 = s["q2"]
                pss = []
                for h in range(n_half):
                    psh = psp.tile([128, mm_n], f32, tag="ps")
                    pss.append(psh)
                for t in range(tj):
                    for h in range(n_half):
                        n0 = h * mm_n
                        nc.tensor.matmul(pss[h][:],
                                         lhsT=xbf3[:, t, :],
                                         rhs=q2_3[:, t, n0:n0 + mm_n],
                                         start=(t == 0),
                                         stop=(t == tj - 1))
                osb = osbp.tile([128, n_loc], f32, tag="osb")
                for h in range(n_half):
                    n0 = h * mm_n
                    nc.scalar.activation(osb[:, n0:n0 + mm_n], pss[h][:],
                                         Act.Copy, scale=0.5)
                nc.vector.tensor_tensor(osb[:], osb[:], biasb_sb[:],
                                        op=Alu.add)
                nc.sync.dma_start(out[mi * 128:(mi + 1) * 128, :], osb[:])

            def emit_prologue(r):
                """Serial prologue (used for iteration 0 only)."""
                for ci in range(n_gch):
                    emit_gamma_wg(r, ci)
                    emit_gamma_red(r, ci)
                emit_gamma_fin1(r)
                emit_gamma_rd(r)
                emit_gamma_fin2(r)
                for ci in range(n_wch):
                    emit_wt(r, ci)
                for ci in range(n_wch):
                    emit_quant(r, ci)

            # Pipeline quanta for iteration r+1, keyed by the m-tile index of
            # iteration r's phase B after which they are emitted.  Dependencies
            # must already be met at that point in each engine's stream (the
            # sequencers are in-order, so a premature wait blocks everything
            # behind it on that engine).
            sched = {}
            for ci in range(n_gch):          # 4 wg chunks / m-tile
                sched.setdefault(pip_wg + ci // 4, []).append(("wg", ci))
            for ci in range(n_gch):
                sched.setdefault(pip_red + ci // 4, []).append(("red", ci))
            span_wt = m_tiles - 6 - pip_wt
            for ci in range(n_wch):          # wt chunks spread to ~m-tile 26
                sched.setdefault(
                    pip_wt + ci * span_wt // n_wch, []).append(("wt", ci))
            span_q = m_tiles - 2 - pip_qnt
            for ci in range(n_wch):          # quantize spread to ~m-tile 30
                sched.setdefault(
                    pip_qnt + ci * span_q // n_wch, []).append(("qnt", ci))
            sched.setdefault(pip_fin1, []).append(("fin1", None))
            sched.setdefault(pip_rd, []).append(("rd", None))
            sched.setdefault(pip_fin2, []).append(("fin2", None))

            emitters = {"wg": emit_gamma_wg, "red": emit_gamma_red,
                        "wt": emit_wt, "qnt": emit_quant,
                        "fin1": lambda r, ci: emit_gamma_fin1(r),
                        "rd": lambda r, ci: emit_gamma_rd(r),
                        "fin2": lambda r, ci: emit_gamma_fin2(r)}

            emit_prologue(0)
            for r in range(rep):
                pipelined = r + 1 < rep
                for mi in range(m_tiles):
                    emit_mtile(r, mi)
                    if pipelined:
                        for what, ci in sched.get(mi, []):
                            emitters[what](r + 1, ci)

    if dedup_ldw:
        dedup_ldweights(nc)
    if split_waits:
        split_multi_waits(nc)
    return nc


def shard_inputs(x, weight, bias, m_loc=M_LOC, n_loc=N_LOC, n_cores=N_CORES,
                 grid_n=GRID_N):
    """Host-side layout prep (transpose/slice/broadcast only)."""
    x2 = np.ascontiguousarray(x.reshape(-1, x.shape[-1]))     # [M, K]
    k = x2.shape[1]
    m_tiles, tj = m_loc // 128, k // 128
    grid_m = n_cores // grid_n
    ng = n_loc // grid_m
    in_maps = []
    xts = {}
    for c in range(n_cores):
        mi, ni = c // grid_n, c % grid_n
        if mi not in xts:
            # xt[mi, p, t*128+j] = x_loc[mi*128+j, t*128+p]
            xl = x2[mi * m_loc:(mi + 1) * m_loc, :]
            xts[mi] = np.ascontiguousarray(
                xl.reshape(m_tiles, 128, tj, 128)
                .transpose(0, 3, 2, 1)
                .reshape(m_tiles, 128, tj * 128))
        wt = np.ascontiguousarray(weight[ni * n_loc:(ni + 1) * n_loc, :].T)
        g0 = ni * n_loc + mi * ng
        wgt = np.ascontiguousarray(weight[g0:g0 + ng, :].T)
        bb = np.ascontiguousarray(
            np.broadcast_to(bias[ni * n_loc:(ni + 1) * n_loc], (128, n_loc)))
        in_maps.append({"xt": xts[mi], "wt": wt, "wg": wgt, "biasb": bb})
    return in_maps


def unshard_output(outs, x_shape, m_loc=M_LOC, n_loc=N_LOC, n_cores=N_CORES,
                   grid_m=GRID_M, grid_n=GRID_N):
    n = grid_n * n_loc
    full = np.empty((grid_m * m_loc, n), dtype=outs[0].dtype)
    for c in range(n_cores):
        mi, ni = c // grid_n, c % grid_n
        full[mi * m_loc:(mi + 1) * m_loc, ni * n_loc:(ni + 1) * n_loc] = outs[c]
    return full.reshape(*x_shape[:-1], n)


def kernel(x, weight, bias):
    from concourse.bass_utils import run_bass_kernel_spmd

    nc = build_nc()
    in_maps = shard_inputs(x, weight, bias)
    res = run_bass_kernel_spmd(nc, in_maps, core_ids=list(range(N_CORES)))
    outs = [res.results[c]["out"] for c in range(N_CORES)]
    return unshard_output(outs, x.shape)


# revision 33
# speedup vs baseline: 1.0417x; 1.0307x over previous
"""BitLinear (BitNet b1.58 ternary-weight linear) Trainium2 kernel, 8-core SPMD.

Reference computation:
    gamma = max(mean(|W|), 1e-8)
    QW    = clip(round(W / gamma), -1, 1)          # in {-1, 0, 1}
    out   = x @ QW.T + bias                        # x: [4, 2048, 4096] f32

Sharding (2 x 4 grid over 8 cores):
    - x   split in half along the (flattened) batch axis M=8192 -> M_loc=4096,
      transposed on host to xT [K, M_loc] so the contraction dim lands on
      SBUF partitions.
    - W   split in 4 along out_features N=4096 -> N_loc=1024, transposed on
      host to wT [K, N_loc].  Each W shard is held by 2 cores (the two m-halves).
    - gamma needs mean(|W|) over the FULL W: each core abs-sums a disjoint
      1/8 slice (wg), a 1-element AllReduce sums across cores.

Per-core dataflow (one iteration):
    1. gamma: stream wg [K, ng] f32, per-k-tile abs-reduce on VectorE,
       cross-partition sum via PE, 1-float AllReduce, th = gamma/2 broadcast
       to [128, 1] via PE.
    2. quantize: stream wT f32; produce q2 = 2*qw in bf16, split by columns
       across two engines:
         - ScalarE half: q2 = Sign(w - th) + Sign(w + th)   (two ACTs + DVE add)
         - VectorE half: q2 = 2*(w > th) - 2*(w < -th)      (two tensor_scalar
           with per-partition th + DVE subtract)
    3. matmul: x m-tiles are DMA-cast to exact bf16 by SWDGE (no engine work);
       out[m, n] = sum_k x[m,k] * q2[n,k] accumulated in f32 PSUM over 32
       k-tiles in two 512-wide halves (one PSUM bank each, 7 half-banks in
       flight).
    4. epilogue: ScalarE copies PSUM->SBUF with scale 0.5 (undoing the 2x in
       q2), VectorE adds bias, DMA out.

With rep > 1 the kernel body is software-pipelined at trace level: iteration
r+1's gamma / AllReduce / quantize are emitted in small quanta between
iteration r's m-tiles, positioned so each quantum's dependencies are already
met when its engine reaches it (the sequencers are in-order, so a premature
wait head-of-line-blocks everything behind it on that engine).  In steady
state the PE then runs m-tile matmuls back to back with no phase bubble.

kernel(**inputs) takes the full unsharded inputs and returns the full output.
Host work is layout only (transpose / slice / broadcast / concat); all
arithmetic runs on the NeuronCores.
"""

import numpy as np

N_CORES = 8
GRID_M, GRID_N = 2, 4          # core c -> (mi, ni) = (c // GRID_N, c % GRID_N)

B, S, K, N = 4, 2048, 4096, 4096
M = B * S                      # 8192
M_LOC = M // GRID_M            # 4096
N_LOC = N // GRID_N            # 1024
TJ = K // 128                  # 32 k-tiles
MM_N = 512                     # matmul moving free dim (one PSUM bank of f32)

TH_FLOOR = 0.5e-8              # = (1e-8)/2, the gamma floor folded into th


def split_multi_waits(nc, limit=1):
    """The walrus build in this container supports only `limit` sync-waits on
    CTRL-type (Drain/NoOp) instructions, but Tile's exit barrier attaches one
    wait per outstanding processor.  Split the extras onto preceding
    single-wait NOPs on the same engine (waits execute in issue order on the
    sequencer, so this is semantically identical)."""
    import concourse.mybir as mybir

    n_split = 0
    for f in nc.m.functions:
        for b in f.blocks:
            out_list = []
            changed = False
            for ins in b.instructions:
                si = getattr(ins, "sync_info", None)
                ow = list(si.on_wait) if (si is not None and si.on_wait) else []
                if len(ow) > limit:
                    for j, w in enumerate(ow[:-limit]):
                        nop = mybir.InstNoOp(name=f"{ins.name}-ws{j}")
                        nop.engine = ins.engine
                        nop.sync_info = mybir.SyncInfo(on_wait=[w], on_update=[])
                        out_list.append(nop)
                        n_split += 1
                    si.on_wait = ow[-limit:]
                    changed = True
                out_list.append(ins)
            if changed:
                b.instructions = out_list
    return n_split


def dedup_ldweights(nc):
    """Tile lowers every matmul into an explicit Ldweights + Matmult pair, so
    two consecutive matmuls sharing one stationary tile reload the PE array
    twice.  Drop an Ldweights when the instruction directly before it is a
    Matmult whose stationary operand is byte-identical and the Ldweights
    carries no semaphore waits/updates — the weights are already in the
    array."""
    n_drop = 0
    for f in nc.m.functions:
        for b in f.blocks:
            insts = list(b.instructions)
            out_list = []
            for ins in insts:
                if (type(ins).__name__ == "InstLdweights"
                        and out_list
                        and type(out_list[-1]).__name__ == "InstMatmult"
                        and len(out_list[-1].ins) >= 2
                        and str(out_list[-1].ins[1]) == str(ins.ins[0])
                        and not (ins.sync_info and ins.sync_info.on_wait)
                        and not (ins.sync_info and ins.sync_info.on_update)):
                    n_drop += 1
                    continue
                out_list.append(ins)
            if n_drop:
                b.instructions = out_list
    return n_drop


def build_nc(m_loc=M_LOC, k=K, n_loc=N_LOC, n_cores=N_CORES,
             grid_m=GRID_M, split_waits=True, rep=1, mm_n=MM_N,
             dedup_ldw=True, n_act=640, gch_tch=1, gch_bufs=4,
             qtmp_bufs=2,
             wch_tch=2, wch_bufs=3, xbf_bufs=3, osb_bufs=2, ps_bufs=7,
             x_cast="dma", gate_loads=True,
             # pipeline emission points (m-tile index in the PREVIOUS
             # iteration's phase B after which each quantum is emitted)
             pip_wg=2, pip_red=3, pip_fin1=11, pip_rd=14, pip_fin2=15,
             pip_wt=12, pip_qnt=16):
    """Build the per-core Bass graph (SPMD: identical on every core).

    rep > 1 unrolls + software-pipelines the kernel body `rep` times (same
    inputs / outputs each time) so steady-state per-iteration HW time can be
    measured without per-dispatch tunnel overhead.  rep=1 is the real kernel.
    """
    import concourse.bass as bass
    import concourse.mybir as mybir
    import concourse.tile as tile

    f32 = mybir.dt.float32
    bf16 = mybir.dt.bfloat16
    Alu = mybir.AluOpType
    Act = mybir.ActivationFunctionType

    tj = k // 128
    m_tiles = m_loc // 128
    ng = n_loc // grid_m            # gamma-slice width (disjoint across cores)
    # AR over the disjoint wg slices sums |W| exactly once; th = gamma/2
    th_scale = 1.0 / (2.0 * k * (n_loc * (n_cores // grid_m)))
    n_half = (n_loc + mm_n - 1) // mm_n
    assert n_loc % mm_n == 0
    n_gch = tj // gch_tch           # gamma chunks
    n_wch = tj // wch_tch           # quantize chunks

    nc = bass.Bass(num_devices=n_cores)
    # xt is host-pre-tiled: xt[mi, p, t*128+j] = x_loc[mi*128+j, t*128+p]
    # so each m-tile's load is one fully-contiguous [128, tj*128] block.
    xt = nc.dram_tensor("xt", [m_tiles, 128, tj * 128], f32,
                        kind="ExternalInput")
    wt = nc.dram_tensor("wt", [k, n_loc], f32, kind="ExternalInput")
    wg = nc.dram_tensor("wg", [k, ng], f32, kind="ExternalInput")
    biasb = nc.dram_tensor("biasb", [128, n_loc], f32, kind="ExternalInput")
    out = nc.dram_tensor("out", [m_loc, n_loc], f32, kind="ExternalOutput")

    cc_in = nc.dram_tensor("cc_in", [1], f32, kind="Internal")
    cc_out = nc.dram_tensor("cc_out", [1], f32, kind="Internal",
                            addr_space="Shared")

    wt_r = wt[:, :].rearrange("(t p) n -> p t n", p=128)
    wg_r = wg[:, :].rearrange("(t p) n -> p t n", p=128)

    with tile.TileContext(nc) as tc:
        with (
            tc.tile_pool(name="const", bufs=1) as constp,
            tc.tile_pool(name="gam", bufs=2) as gamp,
            tc.tile_pool(name="gch", bufs=gch_bufs) as gchp,
            tc.tile_pool(name="wch", bufs=wch_bufs) as wchp,
            tc.tile_pool(name="qtmp", bufs=qtmp_bufs) as qtmpp,
            tc.tile_pool(name="q2", bufs=2) as q2p,
            tc.tile_pool(name="xbf", bufs=xbf_bufs) as xbfp,
            tc.tile_pool(name="osb", bufs=osb_bufs) as osbp,
            tc.tile_pool(name="ps", bufs=ps_bufs, space="PSUM") as psp,
            tc.tile_pool(name="gps", bufs=1, space="PSUM") as gpsp,
        ):
            # ---- constants ----
            biasb_sb = constp.tile([128, n_loc], f32, tag="biasb")
            nc.sync.dma_start(biasb_sb[:], biasb[:, :])
            ones_col = constp.tile([128, 1], f32, tag="ones_col")
            nc.vector.memset(ones_col[:], 1.0)
            ones_row = constp.tile([1, 128], f32, tag="ones_row")
            nc.vector.memset(ones_row[:], 1.0)

            # Per-iteration state carried between emit quanta.
            st = [dict() for _ in range(rep)]
            prev_cc = [None]          # last collective, for cc buffer reuse
            prev_rd = [None]          # last AR readback, for cc_out reuse

            def emit_gamma_wg(r, ci):
                """DMA one wg chunk (gch_tch k-tiles).  The SP ring carries
                only loads (out-stores ride the ACT ring), so these can never
                be head-of-line blocked by a store waiting on an epilogue."""
                s = st[r]
                gch = gchp.tile([128, gch_tch * ng], f32, tag="gch")
                gch3 = gch[:].rearrange("p (t n) -> p t n", n=ng)
                for t2 in range(gch_tch):
                    nc.sync.dma_start(gch3[:, t2, :],
                                      wg_r[:, ci * gch_tch + t2, :])
                s.setdefault("gch", []).append(gch3)

            def emit_gamma_red(r, ci):
                """Abs-reduce one wg chunk into acc[:, ...]."""
                s = st[r]
                if "acc" not in s:
                    acc = gamp.tile([128, tj], f32, tag="acc")
                    s["acc"] = acc
                gch3 = s["gch"][ci]
                for t2 in range(gch_tch):
                    t = ci * gch_tch + t2
                    red = nc.vector.tensor_reduce(
                        s["acc"][:, t:t + 1], gch3[:, t2, :],
                        axis=mybir.AxisListType.X, op=Alu.add,
                        apply_absolute_value=True)
                    if ci == 0 and t2 == 0 and r > 0 \
                            and "th_ins" in st[r - 1]:
                        # Keep this iteration's reduces behind the previous
                        # iteration's th on the in-order DVE stream — the
                        # scheduler otherwise hoists them in front, and a late
                        # AR then stalls th behind 30+ queued reduces.
                        tile.add_dep_helper(
                            red.ins, st[r - 1]["th_ins"].ins,
                            reason="gamma reds yield to previous th")

            def emit_gamma_fin1(r):
                """Final local reduce + ship to the AllReduce."""
                s = st[r]
                acc1 = gamp.tile([128, 1], f32, tag="acc1")
                nc.vector.tensor_reduce(acc1[:], s["acc"][:],
                                        axis=mybir.AxisListType.X, op=Alu.add)
                ps1 = gpsp.tile([128, 1], f32, tag="gps")
                nc.tensor.matmul(ps1[0:1, 0:1], lhsT=acc1[:], rhs=ones_col[:],
                                 start=True, stop=True)
                s_sb = gamp.tile([1, 1], f32, tag="s_sb")
                nc.vector.tensor_copy(s_sb[:], ps1[0:1, 0:1])
                # cc DMAs ride the ACT HWDGE ring (not behind bulk SP loads)
                gate = nc.scalar.dma_start(cc_in[0:1], s_sb[0:1, 0])
                if prev_cc[0] is not None:
                    tile.add_dep_helper(gate.ins, prev_cc[0].ins,
                                        reason="cc_in reuse after prior AR")
                cc = nc.gpsimd.collective_compute(
                    "AllReduce", Alu.add,
                    replica_groups=[list(range(n_cores))],
                    ins=[cc_in.ap().opt()], outs=[cc_out.ap().opt()])
                tile.add_dep_helper(cc.ins, gate.ins, reason="AR reads cc_in")
                if prev_rd[0] is not None:
                    tile.add_dep_helper(cc.ins, prev_rd[0].ins,
                                        reason="cc_out reuse after prior read")
                s["gate"], s["cc"] = gate, cc
                prev_cc[0] = cc

            def emit_gamma_rd(r):
                """Read the AR result back (emitted once the mesh is done so
                the ACT ring is never blocked waiting on it)."""
                s = st[r]
                s2_sb = gamp.tile([1, 1], f32, tag="s2_sb")
                rd = nc.scalar.dma_start(s2_sb[0:1, 0], cc_out[0:1])
                tile.add_dep_helper(rd.ins, s["cc"].ins,
                                    reason="read AR result")
                s["s2_sb"], s["rd"] = s2_sb, rd
                prev_rd[0] = rd

            def emit_gamma_fin2(r):
                """Broadcast the AR result via PE, compute th/nth."""
                s = st[r]
                psb = gpsp.tile([128, 1], f32, tag="gps")
                nc.tensor.matmul(psb[:], lhsT=ones_row[:], rhs=s["s2_sb"][:],
                                 start=True, stop=True)
                th = gamp.tile([128, 1], f32, tag="th")
                nth = gamp.tile([128, 1], f32, tag="nth")
                th_ins = nc.vector.tensor_scalar(
                    th[:], psb[:], th_scale, TH_FLOOR,
                    op0=Alu.mult, op1=Alu.max)
                nc.vector.tensor_scalar(nth[:], psb[:], -th_scale, -TH_FLOOR,
                                        op0=Alu.mult, op1=Alu.min)
                s["th"], s["nth"], s["th_ins"] = th, nth, th_ins

            def emit_wt(r, ci):
                """DMA one wt chunk (wch_tch k-tiles), gated so it can't
                starve the gamma stream of bandwidth."""
                s = st[r]
                wchs = s.setdefault("wch", {})
                wch = wchp.tile([128, wch_tch * n_loc], f32, tag="wg")
                wch3 = wch[:].rearrange("p (t n) -> p t n", n=n_loc)
                for t2 in range(wch_tch):
                    wd = nc.sync.dma_start(
                        wch3[:, t2, :], wt_r[:, ci * wch_tch + t2, :])
                    if gate_loads:
                        tile.add_dep_helper(
                            wd.ins, s["gate"].ins,
                            reason="wt loads yield to gamma stream")
                wchs[ci] = wch3

            def emit_quant(r, ci):
                """Quantize one wt chunk into q2 (split ScalarE / VectorE)."""
                s = st[r]
                if "q2" not in s:
                    q2 = q2p.tile([128, tj * n_loc], bf16, tag="q2")
                    s["q2"] = q2[:].rearrange("p (t n) -> p t n", n=n_loc)
                q2_3 = s["q2"]
                th, nth = s["th"], s["nth"]
                wch3 = s["wch"].pop(ci)
                n_dve = n_loc - n_act
                for t2 in range(wch_tch):
                    t = ci * wch_tch + t2
                    if n_act > 0:
                        a = qtmpp.tile([128, n_act], bf16, tag="qa")
                        b = qtmpp.tile([128, n_act], bf16, tag="qb")
                        nc.scalar.activation(
                            a[:], wch3[:, t2, 0:n_act], Act.Sign,
                            bias=nth[:], scale=1.0)
                        nc.scalar.activation(
                            b[:], wch3[:, t2, 0:n_act], Act.Sign,
                            bias=th[:], scale=1.0)
                        nc.vector.tensor_tensor(
                            q2_3[:, t, 0:n_act], a[:], b[:], op=Alu.add)
                    if n_dve > 0:
                        p2 = qtmpp.tile([128, n_dve], bf16, tag="qp")
                        m2 = qtmpp.tile([128, n_dve], bf16, tag="qm")
                        nc.vector.tensor_scalar(
                            p2[:], wch3[:, t2, n_act:], th[:], 2.0,
                            op0=Alu.is_gt, op1=Alu.mult)
                        nc.vector.tensor_scalar(
                            m2[:], wch3[:, t2, n_act:], nth[:], 2.0,
                            op0=Alu.is_lt, op1=Alu.mult)
                        nc.vector.tensor_tensor(
                            q2_3[:, t, n_act:], p2[:], m2[:], op=Alu.subtract)

            def emit_mtile(r, mi):
                """One m-tile of phase B: x load, 64 matmuls, epilogue."""
                s = st[r]
                xbf = xbfp.tile([128, tj * 128], bf16, tag="xbf")
                xbf3 = xbf[:].rearrange("p (t j) -> p t j", j=128)
                if x_cast == "dma":
                    xd = nc.gpsimd.dma_start(xbf[:], xt[mi, :, :])
                else:
                    xraw = xbfp.tile([128, tj * 128], f32, tag="xraw")
                    xd = nc.sync.dma_start(xraw[:], xt[mi, :, :])
                    nc.scalar.activation(xbf[:], xraw[:], Act.Copy, scale=1.0)
                if gate_loads:
                    tile.add_dep_helper(xd.ins, s["gate"].ins,
                                        reason="x loads yield to gamma stream")
                q2_3 = s["q2"]
                pss = []
                for h in range(n_half):
                    psh = psp.tile([128, mm_n], f32, tag="ps")
                    pss.append(psh)
                for t in range(tj):
                    for h in range(n_half):
                        n0 = h * mm_n
                        nc.tensor.matmul(pss[h][:],
                                         lhsT=xbf3[:, t, :],
                                         rhs=q2_3[:, t, n0:n0 + mm_n],
                                         start=(t == 0),
                                         stop=(t == tj - 1))
                osb = osbp.tile([128, n_loc], f32, tag="osb")
                for h in range(n_half):
                    n0 = h * mm_n
                    nc.scalar.activation(osb[:, n0:n0 + mm_n], pss[h][:],
                                         Act.Copy, scale=0.5)
                nc.vector.tensor_tensor(osb[:], osb[:], biasb_sb[:],
                                        op=Alu.add)
                nc.sync.dma_start(out[mi * 128:(mi + 1) * 128, :], osb[:])

            def emit_prologue(r):
                """Serial prologue (used for iteration 0 only)."""
                for ci in range(n_gch):
                    emit_gamma_wg(r, ci)
                    emit_gamma_red(r, ci)
                emit_gamma_fin1(r)
                emit_gamma_rd(r)
                emit_gamma_fin2(r)
                for ci in range(n_wch):
                    emit_wt(r, ci)
                for ci in range(n_wch):
                    emit_quant(r, ci)

            # Pipeline quanta for iteration r+1, keyed by the m-tile index of
            # iteration r's phase B after which they are emitted.  Dependencies
            # must already be met at that point in each engine's stream (the
            # sequencers are in-order, so a premature wait blocks everything
            # behind it on that engine).
            sched = {}
            for ci in range(n_gch):          # 4 wg chunks / m-tile
                sched.setdefault(pip_wg + ci // 4, []).append(("wg", ci))
            for ci in range(n_gch):
                sched.setdefault(pip_red + ci // 4, []).append(("red", ci))
            span_wt = m_tiles - 6 - pip_wt
            for ci in range(n_wch):          # wt chunks spread to ~m-tile 26
                sched.setdefault(
                    pip_wt + ci * span_wt // n_wch, []).append(("wt", ci))
            span_q = m_tiles - 2 - pip_qnt
            for ci in range(n_wch):          # quantize spread to ~m-tile 30
                sched.setdefault(
                    pip_qnt + ci * span_q // n_wch, []).append(("qnt", ci))
            sched.setdefault(pip_fin1, []).append(("fin1", None))
            sched.setdefault(pip_rd, []).append(("rd", None))
            sched.setdefault(pip_fin2, []).append(("fin2", None))

            emitters = {"wg": emit_gamma_wg, "red": emit_gamma_red,
                        "wt": emit_wt, "qnt": emit_quant,
                        "fin1": lambda r, ci: emit_gamma_fin1(r),
                        "rd": lambda r, ci: emit_gamma_rd(r),
                        "fin2": lambda r, ci: emit_gamma_fin2(r)}

            emit_prologue(0)
            for r in range(rep):
                pipelined = r + 1 < rep
                for mi in range(m_tiles):
                    emit_mtile(r, mi)
                    if pipelined:
                        for what, ci in sched.get(mi, []):
                            emitters[what](r + 1, ci)

    if dedup_ldw:
        dedup_ldweights(nc)
    if split_waits:
        split_multi_waits(nc)
    return nc


def shard_inputs(x, weight, bias, m_loc=M_LOC, n_loc=N_LOC, n_cores=N_CORES,
                 grid_n=GRID_N):
    """Host-side layout prep (transpose/slice/broadcast only)."""
    x2 = np.ascontiguousarray(x.reshape(-1, x.shape[-1]))     # [M, K]
    k = x2.shape[1]
    m_tiles, tj = m_loc // 128, k // 128
    grid_m = n_cores // grid_n
    ng = n_loc // grid_m
    in_maps = []
    xts = {}
    for c in range(n_cores):
        mi, ni = c // grid_n, c % grid_n
        if mi not in xts:
            # xt[mi, p, t*128+j] = x_loc[mi*128+j, t*128+p]
            xl = x2[mi * m_loc:(mi + 1) * m_loc, :]
            xts[mi] = np.ascontiguousarray(
                xl.reshape(m_tiles, 128, tj, 128)
                .transpose(0, 3, 2, 1)
                .reshape(m_tiles, 128, tj * 128))
        wt = np.ascontiguousarray(weight[ni * n_loc:(ni + 1) * n_loc, :].T)
        g0 = ni * n_loc + mi * ng
        wgt = np.ascontiguousarray(weight[g0:g0 + ng, :].T)
        bb = np.ascontiguousarray(
            np.broadcast_to(bias[ni * n_loc:(ni + 1) * n_loc], (128, n_loc)))
        in_maps.append({"xt": xts[mi], "wt": wt, "wg": wgt, "biasb": bb})
    return in_maps


def unshard_output(outs, x_shape, m_loc=M_LOC, n_loc=N_LOC, n_cores=N_CORES,
                   grid_m=GRID_M, grid_n=GRID_N):
    n = grid_n * n_loc
    full = np.empty((grid_m * m_loc, n), dtype=outs[0].dtype)
    for c in range(n_cores):
        mi, ni = c // grid_n, c % grid_n
        full[mi * m_loc:(mi + 1) * m_loc, ni * n_loc:(ni + 1) * n_loc] = outs[c]
    return full.reshape(*x_shape[:-1], n)


def kernel(x, weight, bias):
    from concourse.bass_utils import run_bass_kernel_spmd

    nc = build_nc()
    in_maps = shard_inputs(x, weight, bias)
    res = run_bass_kernel_spmd(nc, in_maps, core_ids=list(range(N_CORES)))
    outs = [res.results[c]["out"] for c in range(N_CORES)]
    return unshard_output(outs, x.shape)
